# revision 1
# baseline (speedup 1.0000x reference)
"""LlamaAttention (B=2, S=2048, H=4096, NH=32) on 8 Trainium2 NeuronCores.

Sharding: tensor-parallel over heads (4 heads / core). Column-parallel
Wq/Wk/Wv, row-parallel Wo; the Wo partial sums are reduced on the host
(the all-reduce of the TP recipe, done during unshard).

Per-core dataflow (all matmuls fp32r = full-rate reduced-precision fp32):
  per batch b:
    phase 1: Q^T,K^T = RoPE(W^T-chunk @ X^T-chunk) -> DRAM  [d, t] layout
             V       = X^T-chunk^T @ WvT           -> DRAM  [t, d] layout
    phase 2: per head: S^T[k,q] = K^T-tile^T @ Q^T  (contraction d)
             exp on ACT; denominators via ones-matmul (partition-broadcast
             column sums); ctx^T[d,q] = V-tile^T @ expS^T over k tiles.
             Only non-fully-masked 128x512 score blocks are computed.
  phase 3: O^T partial = WoT-tile^T @ ctx^T -> DRAM [o, t] layout

Host side: pre-transposes X and the weights (layout marshaling), builds
the block structure from the attention mask, sums the 8 partial O^T
outputs and transposes back.
"""
import sys

sys.path.insert(0, "/opt/trn_rl_repo")

import numpy as np

import concourse.bass as bass
import concourse.bacc as bacc
import concourse.tile as tile
import concourse.mybir as mybir

B, S, H, NH = 2, 2048, 4096, 32
HD = H // NH          # 128
NC = 8                # cores
DL = H // NC          # 512 local dims (4 heads / core)
NHL = NH // NC        # 4 local heads
BT = B * S            # 4096 tokens
P = 128
SLICE = 1024          # phase-1 token slice (W chunks reused across it)
CH = 512              # phase-1 X^T chunk (matmul moving dim)
QT = 512              # phase-2 query tile (free dim)
KT = 128              # phase-2 key tile (partition dim)
NKO = H // P          # 32 contraction subtiles

DT = mybir.dt.float32
DTR = mybir.dt.float32r
F32 = mybir.dt.float32
AF = mybir.ActivationFunctionType


def _phase1_batch(nc, tc, b, pools, aps, scratch):
    """QKV projections + RoPE for batch b."""
    p1, p1t, p1w, p1s, p1r, psA, psV = pools
    xt3, wq3, wk3, wv3, cosq, sinq, cosk, sink = aps
    qt_d, kt_d, v_d = scratch          # per-batch tiles [DL, S], [DL, S], [S, DL]

    for sl in range(S // SLICE):                       # 2 slices per batch
        t0 = b * S + sl * SLICE                        # global token offset
        xch = []
        for c in range(SLICE // CH):                   # 2 chunks
            xc = p1.tile([P, NKO, CH], DTR, tag="xt", name=f"xt{c}")
            nc.sync.dma_start(xc[:], xt3[:, :, bass.ds(t0 + c * CH, CH)])
            xch.append(xc)
        tabs = {}
        for nm, t_ap in (("cq", cosq), ("sq", sinq), ("ck", cosk), ("sk", sink)):
            tt = p1t.tile([P, SLICE], DT, tag="tab_" + nm)
            nc.sync.dma_start(tt[:], t_ap[:, bass.ds(sl * SLICE, SLICE)])
            tabs[nm] = tt
        # --- Q^T and K^T with RoPE ---
        for (w3, cnm, snm, outd) in ((wq3, "cq", "sq", qt_d),
                                     (wk3, "ck", "sk", kt_d)):
            cosT, sinT = tabs[cnm], tabs[snm]
            for dsub in range(DL // P):
                w_sb = p1w.tile([P, NKO, P], DTR, tag="wqk")
                nc.sync.dma_start(w_sb[:], w3[:, :, bass.ts(dsub, P)])
                for c in range(SLICE // CH):
                    psum = psA.tile([P, CH], F32, tag="qk")
                    for hs in range(NKO):
                        nc.tensor.matmul(
                            psum[:], w_sb[:, hs, :], xch[c][:, hs, :],
                            start=(hs == 0), stop=(hs == NKO - 1))
                    csl = bass.ds(sl * SLICE + c * CH, CH)
                    tsl = bass.ds(c * CH, CH)
                    rc = p1r.tile([P, CH], DTR, tag="rc")
                    rs = p1r.tile([P, CH], F32, tag="rs")
                    nc.vector.tensor_mul(rc[:], psum[:], cosT[:, tsl])
                    nc.vector.tensor_mul(
                        rs[0:64, :], psum[64:128, :], sinT[0:64, tsl])
                    nc.vector.tensor_mul(
                        rs[64:128, :], psum[0:64, :], sinT[64:128, tsl])
                    nc.vector.tensor_tensor(
                        rc[0:64, :], rc[0:64, :], rs[0:64, :],
                        mybir.AluOpType.subtract)
                    nc.vector.tensor_tensor(
                        rc[64:128, :], rc[64:128, :], rs[64:128, :],
                        mybir.AluOpType.add)
                    nc.sync.dma_start(outd[bass.ts(dsub, P), csl], rc[:])
        # --- V in [t, d] layout; waves in reverse chunk order so the
        # first chunk's slot frees early for the next slice's prefetch ---
        for c in reversed(range(SLICE // CH)):
            psums = [psV.tile([P, DL], F32, tag="v", name=f"vps{j}")
                     for j in range(CH // P)]
            for hs in range(NKO):
                wv_sb = p1s.tile([P, DL], DTR, tag="wv")
                nc.sync.dma_start(wv_sb[:], wv3[:, hs, :])
                for j in range(CH // P):
                    nc.tensor.matmul(
                        psums[j][:], xch[c][:, hs, bass.ts(j, P)], wv_sb[:],
                        start=(hs == 0), stop=(hs == NKO - 1))
            for j in range(CH // P):
                vo = p1s.tile([P, DL], DTR, tag="vo")
                nc.vector.tensor_copy(vo[:], psums[j][:])
                nc.sync.dma_start(
                    v_d[bass.ds(sl * SLICE + c * CH + j * P, P), :], vo[:])


def _phase2_batch(nc, tc, b, spec, pools, maskt, mb, ones_r, scratch, ctxT):
    """Attention for batch b -> ctxT [P, NHL, S]."""
    p2, p2e, p2m, psS, psSum, psC = pools
    qt_d, kt_d, v_d = scratch

    for h in range(NHL):
        k_sb = p2.tile([P, S], DTR, tag="k_sb")
        nc.sync.dma_start(k_sb[:], kt_d[bass.ts(h, P), :])
        q_sb = p2.tile([P, S], DTR, tag="q_sb")
        nc.sync.dma_start(q_sb[:], qt_d[bass.ts(h, P), :])
        v_sb = p2.tile([P, S // P, P], DTR, tag="v_sb")
        nc.sync.dma_start(
            v_sb[:], v_d[:, bass.ts(h, P)].rearrange("(kt p) d -> p kt d", p=P))
        for qt in range(S // QT):
            blocks = spec[qt]
            nb = len(blocks)
            psum_sum = psSum.tile([P, QT], F32, tag="sum")
            psum_ctx = psC.tile([P, QT], F32, tag="ctx")
            for bi, (kt, masked) in enumerate(blocks):
                psum_s = psS.tile([P, QT], F32, tag="s")
                nc.tensor.matmul(
                    psum_s[:], k_sb[:, bass.ts(kt, KT)],
                    q_sb[:, bass.ts(qt, QT)], start=True, stop=True)
                if masked:
                    mk = p2m.tile([P, QT], DT, tag="mk")
                    nc.sync.dma_start(
                        mk[:], maskt[mb, bass.ts(kt, KT), bass.ts(qt, QT)])
                    nc.vector.tensor_tensor(
                        psum_s[:], psum_s[:], mk[:], mybir.AluOpType.add)
                e_sb = p2e.tile([P, QT], DTR, tag="e")
                nc.scalar.activation(e_sb[:], psum_s[:], AF.Exp)
                nc.tensor.matmul(psum_sum[:], ones_r[:], e_sb[:],
                                 start=(bi == 0), stop=(bi == nb - 1))
                nc.tensor.matmul(psum_ctx[:], v_sb[:, kt, :], e_sb[:],
                                 start=(bi == 0), stop=(bi == nb - 1))
            recip = p2e.tile([P, QT], F32, tag="recip")
            nc.vector.reciprocal(recip[:], psum_sum[:])
            nc.vector.tensor_mul(
                ctxT[:, h, bass.ts(qt, QT)], psum_ctx[:], recip[:])


def _phase3(nc, tc, pools, wo3, ctx_tiles, ot):
    p3w, p3o, psO = pools
    for b in range(B):
        ctxT = ctx_tiles[b]
        for oi in range(H // P):
            wo_sb = p3w.tile([P, NHL, P], DTR, tag="wo")
            nc.sync.dma_start(wo_sb[:], wo3[:, :, bass.ts(oi, P)])
            for qt in range(S // QT):
                psum_o = psO.tile([P, QT], F32, tag="o")
                for hs in range(NHL):
                    nc.tensor.matmul(
                        psum_o[:], wo_sb[:, hs, :], ctxT[:, hs, bass.ts(qt, QT)],
                        start=(hs == 0), stop=(hs == NHL - 1))
                o_sb = p3o.tile([P, QT], DT, tag="o_sb")
                nc.vector.tensor_copy(o_sb[:], psum_o[:])
                nc.sync.dma_start(
                    ot[bass.ts(oi, P), bass.ds(b * S + qt * QT, QT)], o_sb[:])


def _build(specs, n_mb, reps=1, phases=(1, 2, 3)):
    nc = bacc.Bacc()

    xt = nc.declare_dram_parameter("xt", [H, BT], DTR, isOutput=False)
    wqt = nc.declare_dram_parameter("wqt", [H, DL], DTR, isOutput=False)
    wkt = nc.declare_dram_parameter("wkt", [H, DL], DTR, isOutput=False)
    wvt = nc.declare_dram_parameter("wvt", [H, DL], DTR, isOutput=False)
    wot = nc.declare_dram_parameter("wot", [DL, H], DTR, isOutput=False)
    maskt = nc.declare_dram_parameter("maskt", [n_mb, S, S], DT, isOutput=False)
    cosq = nc.declare_dram_parameter("cosq", [HD, S], DT, isOutput=False)
    sinq = nc.declare_dram_parameter("sinq", [HD, S], DT, isOutput=False)
    cosk = nc.declare_dram_parameter("cosk", [HD, S], DT, isOutput=False)
    sink = nc.declare_dram_parameter("sink", [HD, S], DT, isOutput=False)
    ot = nc.declare_dram_parameter("ot", [H, BT], DT, isOutput=True)

    xt3 = xt.rearrange("(ho p) t -> p ho t", p=P)
    wq3 = wqt.rearrange("(ho p) d -> p ho d", p=P)
    wk3 = wkt.rearrange("(ho p) d -> p ho d", p=P)
    wv3 = wvt.rearrange("(ho p) d -> p ho d", p=P)
    wo3 = wot.rearrange("(hs p) o -> p hs o", p=P)

    import contextlib

    with tile.TileContext(nc) as tc:
        with (
            tc.tile_pool(name="glob", bufs=1) as glob,
            tc.tile_pool(name="dram", bufs=1, space="DRAM") as dram,
        ):
            scratches = []
            for b in range(B):
                qd = dram.tile([DL, S], DTR, tag=f"qt_d{b}", name=f"qt_d{b}")
                kd = dram.tile([DL, S], DTR, tag=f"kt_d{b}", name=f"kt_d{b}")
                vd = dram.tile([S, DL], DTR, tag=f"v_d{b}", name=f"v_d{b}")
                scratches.append((qd, kd, vd))

            ones_f = glob.tile([P, P], F32, tag="ones_f")
            nc.any.memset(ones_f[:], 1.0)
            ones_r = glob.tile([P, P], DTR, tag="ones_r")
            nc.vector.tensor_copy(ones_r[:], ones_f[:])

            loop_cm = tc.For_i(0, reps, 1) if reps > 1 else contextlib.nullcontext()
            with loop_cm:
                aps = (xt3, wq3, wk3, wv3, cosq, sinq, cosk, sink)
                if 1 in phases:
                    with (
                        tc.tile_pool(name="p1", bufs=2) as p1,
                        tc.tile_pool(name="p1t", bufs=1) as p1t,
                        tc.tile_pool(name="p1w", bufs=2) as p1w,
                        tc.tile_pool(name="p1s", bufs=3) as p1s,
                        tc.tile_pool(name="p1r", bufs=2) as p1r,
                        tc.tile_pool(name="psA", bufs=2, space="PSUM") as psA,
                        tc.tile_pool(name="psV", bufs=4, space="PSUM") as psV,
                    ):
                        p1pools = (p1, p1t, p1w, p1s, p1r, psA, psV)
                        for b in range(B):
                            _phase1_batch(nc, tc, b, p1pools, aps, scratches[b])
                if 2 in phases:
                    with tc.tile_pool(name="ctxp", bufs=1) as ctxp:
                        ctx_tiles = []
                        with (
                            tc.tile_pool(name="p2", bufs=2) as p2,
                            tc.tile_pool(name="p2e", bufs=3) as p2e,
                            tc.tile_pool(name="p2m", bufs=2) as p2m,
                            tc.tile_pool(name="psS", bufs=3, space="PSUM") as psS,
                            tc.tile_pool(name="psSum", bufs=2, space="PSUM") as psSum,
                            tc.tile_pool(name="psC", bufs=2, space="PSUM") as psC,
                        ):
                            for b in range(B):
                                mb = b % n_mb
                                ctxT = ctxp.tile([P, NHL, S], DTR, tag=f"ctxT{b}",
                                                 name=f"ctxT{b}")
                                ctx_tiles.append(ctxT)
                                _phase2_batch(
                                    nc, tc, b, specs[mb],
                                    (p2, p2e, p2m, psS, psSum, psC),
                                    maskt, mb, ones_r, scratches[b], ctxT)
                        if 3 in phases:
                            with (
                                tc.tile_pool(name="p3w", bufs=3) as p3w,
                                tc.tile_pool(name="p3o", bufs=4) as p3o,
                                tc.tile_pool(name="psO", bufs=4, space="PSUM") as psO,
                            ):
                                _phase3(nc, tc, (p3w, p3o, psO), wo3, ctx_tiles, ot)
    nc.finalize()
    return nc


def _rope_tables():
    inv_freq = 1.0 / (10000.0 ** (np.arange(0, HD, 2, dtype=np.float32) / HD))
    t = np.arange(S, dtype=np.float32)
    freqs = np.einsum("i,j->ij", t, inv_freq)
    emb = np.concatenate([freqs, freqs], axis=-1)        # [S, HD]
    return np.cos(emb).astype(np.float32), np.sin(emb).astype(np.float32)


def _block_spec(mask):
    """mask: [S, S] additive mask (q, k). Returns per-qt list of (kt, masked)."""
    spec = []
    for qt in range(S // QT):
        row = []
        sub_q = mask[qt * QT:(qt + 1) * QT]
        for kt in range(S // KT):
            blk = sub_q[:, kt * KT:(kt + 1) * KT]
            if np.all(blk <= -1e8):
                continue                        # fully masked -> skip
            masked = bool(np.any(blk != 0.0))
            row.append((kt, masked))
        assert row, "a query tile with all keys masked is not supported"
        spec.append(row)
    return spec


_CACHE = {}


def kernel(hidden_states, attention_mask, Wq, Wk, Wv, Wo):
    from concourse.bass_utils import run_bass_kernel_spmd

    hidden_states = np.asarray(hidden_states, dtype=np.float32)
    attention_mask = np.asarray(attention_mask, dtype=np.float32)
    Wq = np.asarray(Wq, dtype=np.float32)
    Wk = np.asarray(Wk, dtype=np.float32)
    Wv = np.asarray(Wv, dtype=np.float32)
    Wo = np.asarray(Wo, dtype=np.float32)

    xt = np.ascontiguousarray(hidden_states.reshape(BT, H).T)   # [H, BT]
    wqT = np.ascontiguousarray(Wq.T)                            # [H, H] (in, out)
    wkT = np.ascontiguousarray(Wk.T)
    wvT = np.ascontiguousarray(Wv.T)
    woT = np.ascontiguousarray(Wo.T)                            # [H(in'), H(out)]

    masks = attention_mask[:, 0]                                # [B, S, S]
    same = bool(np.array_equal(masks[0], masks[1])) if B == 2 else True
    n_mb = 1 if same else B
    specs = [_block_spec(masks[i]) for i in range(n_mb)]
    maskt = np.ascontiguousarray(
        np.stack([masks[i].T for i in range(n_mb)]))            # [n_mb, S(k), S(q)]

    cos, sin = _rope_tables()
    scale = 1.0 / np.sqrt(np.float32(HD))
    cosq = np.ascontiguousarray((cos * scale).T)                # [HD, S]
    sinq = np.ascontiguousarray((sin * scale).T)
    cosk = np.ascontiguousarray(cos.T)
    sink = np.ascontiguousarray(sin.T)

    key = (n_mb, tuple(tuple(map(tuple, s)) for s in specs))
    if key not in _CACHE:
        _CACHE[key] = _build(specs, n_mb)
    nc = _CACHE[key]

    in_maps = []
    for g in range(NC):
        dsl = slice(g * DL, (g + 1) * DL)
        in_maps.append({
            "xt": xt,
            "wqt": np.ascontiguousarray(wqT[:, dsl]),
            "wkt": np.ascontiguousarray(wkT[:, dsl]),
            "wvt": np.ascontiguousarray(wvT[:, dsl]),
            "wot": np.ascontiguousarray(woT[dsl, :]),
            "maskt": maskt,
            "cosq": cosq, "sinq": sinq, "cosk": cosk, "sink": sink,
        })

    try:
        res = run_bass_kernel_spmd(nc, in_maps, list(range(NC)), trace=False)
    except Exception:
        # one retry: a wedged NeuronCore usually recovers on re-dispatch
        import time as _time
        _time.sleep(5)
        res = run_bass_kernel_spmd(nc, in_maps, list(range(NC)), trace=False)
    acc = np.zeros((H, BT), dtype=np.float32)
    for g in range(NC):
        acc += res.results[g]["ot"]
    return np.ascontiguousarray(acc.T).reshape(B, S, H)



# revision 2
# speedup vs baseline: 1.0224x; 1.0224x over previous
"""LlamaAttention (B=2, S=2048, H=4096, NH=32) on 8 Trainium2 NeuronCores.

Sharding: tensor-parallel over heads (4 heads / core). Column-parallel
Wq/Wk/Wv, row-parallel Wo; the Wo partial sums are reduced on the host.

v2 design (vs baseline): weights are pre-cast to bf16 on the HOST and kept
RESIDENT in SBUF for all of phase 1 (no per-slice reloads); x is pre-cast
to bf16 on the host (halves streaming); the causal mask is applied from a
couple of small resident patterns instead of 32 MB of mask DMA; the score
scale 1/sqrt(HD) is folded into Wq on the host; phase-2 exp is
software-pipelined one block ahead so the PE never waits on ACT.

Per-core dataflow (all matmuls bf16 x bf16 -> f32 PSUM; exp output is
bf16 so numerator and denominator share the same rounding):
  phase 1 (8 chunks of 512 tokens):
      Q^T,K^T = RoPE(W^T @ X^T-chunk) -> DRAM [d, t] bf16
      V       = X^T-chunk^T @ WvT     -> DRAM [t, d] bf16
  phase 2 per (batch, head): S^T[k,q] = K-tile^T @ Q (contraction d=128),
      exp on ACT (block-pipelined); denominators via ones-matmul
      (partition-broadcast column sums); ctx^T[d,q] = V-tile^T @ expS^T.
      Only not-fully-masked 128x256 blocks are computed; partially masked
      blocks add a resident mask pattern.
  phase 3: O^T partial = WoT^T @ ctx^T -> DRAM [o, t] f32.

Host side: bf16 pre-casts, transposes, mask pattern extraction, sums the
8 partial O^T outputs and transposes back.
"""
import sys

sys.path.insert(0, "/opt/trn_rl_repo")

import numpy as np

import concourse.bass as bass
import concourse.bacc as bacc
import concourse.tile as tile
import concourse.mybir as mybir

B, S, H, NH = 2, 2048, 4096, 32
HD = H // NH          # 128
NC = 8                # cores
DL = H // NC          # 512 local dims (4 heads / core)
NHL = NH // NC        # 4 local heads
BT = B * S            # 4096 tokens
P = 128
CH = 512              # phase-1 x chunk (tokens)
QT = 256              # phase-2 query tile (free dim)
KT = 128              # phase-2 key tile (partition dim)
NKO = H // P          # 32 contraction subtiles
NW = 8                # weight load pieces (hs-sliced)

DT = mybir.dt.float32
DTR = mybir.dt.float32r
BF = mybir.dt.bfloat16
F32 = mybir.dt.float32
AF = mybir.ActivationFunctionType


def _phase1(nc, tc, pools, aps, q_d, k_d, v_d):
    (wpool, xpool, tabpool, rpool, stpool, vstpool, psA, psV) = pools
    xt3, wq3, wk3, wv3, cost, sint = aps

    HS = NKO // NW
    # startup order: interleave x-chunk-0 pieces with wv pieces so the PE
    # can start on chunk 0's V matmuls (which need no tables) within a few
    # microseconds; tables and wq/wk stream in under chunk 0's V compute.
    xc0 = xpool.tile([P, NKO, CH], BF, tag="xt", name="xc0")
    wps = {"wq": [], "wk": [], "wv": []}
    for i in range(NW):
        nc.sync.dma_start(xc0[:, bass.ts(i, HS), :],
                          xt3[:, bass.ts(i, HS), bass.ds(0, CH)])
        t = wpool.tile([P, HS, DL], BF, tag=f"wv{i}", name=f"wv{i}")
        nc.sync.dma_start(t[:], wv3[:, bass.ts(i, HS), :])
        wps["wv"].append(t)
    cosT = tabpool.tile([P, S], BF, tag="cosT")
    nc.sync.dma_start(cosT[:], cost[:, :])
    sinT = tabpool.tile([P, S], BF, tag="sinT")
    nc.sync.dma_start(sinT[:], sint[:, :])
    for nm, w3 in (("wq", wq3), ("wk", wk3)):
        for i in range(NW):
            t = wpool.tile([P, HS, DL], BF, tag=f"{nm}{i}", name=f"{nm}{i}")
            nc.sync.dma_start(t[:], w3[:, bass.ts(i, HS), :])
            wps[nm].append(t)

    xch = xc0
    for c in range(BT // CH):
        cb = (c * CH) // S                  # batch this chunk belongs to
        co = (c * CH) % S                   # column offset within batch
        tsl = bass.ds(co, CH)
        # --- V in [t, d] layout ---
        w_p = wps["wv"]
        vst = vstpool.tile([P, CH // P, DL], BF, tag="vst")
        psums = [psV.tile([P, DL], F32, tag="v", name=f"vps{c}_{j}")
                 for j in range(CH // P)]
        for hs in range(NKO):
            for j in range(CH // P):
                nc.tensor.matmul(
                    psums[j][:], xch[:, hs, bass.ts(j, P)],
                    w_p[hs // HS][:, hs % HS, :],
                    start=(hs == 0), stop=(hs == NKO - 1))
        for j in range(CH // P):
            nc.scalar.activation(vst[:, j, :], psums[j][:], AF.Copy)
        nc.sync.dma_start(
            v_d[cb][bass.ds(co, CH), :].rearrange("(j p) d -> p j d", p=P),
            vst[:])
        # next chunk prefetch, after the V loads/stores of this chunk
        if c + 1 < BT // CH:
            xn = xpool.tile([P, NKO, CH], BF, tag="xt", name=f"xc{c+1}")
            nc.sync.dma_start(xn[:], xt3[:, :, bass.ds((c + 1) * CH, CH)])
        else:
            xn = None
        # --- Q^T and K^T with RoPE ---
        for (nm, outd) in (("wq", q_d[cb]), ("wk", k_d[cb])):
            w_p = wps[nm]
            qst = stpool.tile([P, DL // P, CH], BF, tag="qst")
            for dsub in range(DL // P):
                psum = psA.tile([P, CH], F32, tag="qk")
                for hs in range(NKO):
                    nc.tensor.matmul(
                        psum[:], w_p[hs // HS][:, hs % HS, bass.ts(dsub, P)],
                        xch[:, hs, :], start=(hs == 0), stop=(hs == NKO - 1))
                rc = rpool.tile([P, CH], F32, tag="rc")
                rs = rpool.tile([P, CH], F32, tag="rs")
                nc.vector.tensor_mul(rc[:], psum[:], cosT[:, tsl])
                nc.vector.tensor_mul(rs[0:64, :], psum[64:128, :], sinT[0:64, tsl])
                nc.vector.tensor_mul(rs[64:128, :], psum[0:64, :], sinT[64:128, tsl])
                nc.vector.tensor_tensor(
                    qst[0:64, dsub, :], rc[0:64, :], rs[0:64, :],
                    mybir.AluOpType.subtract)
                nc.vector.tensor_tensor(
                    qst[64:128, dsub, :], rc[64:128, :], rs[64:128, :],
                    mybir.AluOpType.add)
            nc.sync.dma_start(
                outd[:, tsl].rearrange("(ds p) t -> p ds t", p=P), qst[:])
        xch = xn


PIPE = 2  # exp pipeline depth, in block PAIRS (ACT latency ~0.6us ~= 3 scores)


def _phase2(nc, tc, specs, pools, mks, ones_r, q_d, k_d, v_d, ctx_tiles,
            after_loads=None):
    """All (batch, head) attention with one exp pipeline flattened across
    qt/head/batch boundaries, so the PE never drains waiting on ACT.
    Adjacent kt blocks are PAIRED into one [P, 2, QT] PSUM bank so a single
    exp covers both (the ACT fixed access cost ~185ns is per instruction)."""
    (pk, pq, pv, pe_, pr, psS, psSum, psC) = pools

    pend = []  # (e_sb, v_sb, (kt0, kt1), psum_sum, psum_ctx, start, stop, fini)

    def pop_one():
        e_sb, v_sb, kts, ps_sum, ps_ctx, st, sp, fini = pend.pop(0)
        for i, kt in enumerate(kts):
            nc.tensor.matmul(ps_sum[:], ones_r[:], e_sb[:, i, :],
                             start=(st and i == 0), stop=(sp and i == 1))
            nc.tensor.matmul(ps_ctx[:], v_sb[:, kt, :], e_sb[:, i, :],
                             start=(st and i == 0), stop=(sp and i == 1))
        if fini is not None:
            fini()

    for b in range(B):
        spec = specs[b % len(specs)]
        ctxT = ctx_tiles[b]
        for h in range(NHL):
            k_sb = pk.tile([P, S], BF, tag="k_sb")
            for i in range(2):
                nc.sync.dma_start(k_sb[:, bass.ts(i, S // 2)],
                                  k_d[b][bass.ts(h, P), bass.ts(i, S // 2)])
            q_sb = pq.tile([P, S], BF, tag="q_sb")
            for i in range(2):
                nc.sync.dma_start(q_sb[:, bass.ts(i, S // 2)],
                                  q_d[b][bass.ts(h, P), bass.ts(i, S // 2)])
            v_sb = pv.tile([P, S // P, P], BF, tag="v_sb")
            vv = v_d[b][:, bass.ts(h, P)].rearrange("(kt p) d -> p kt d", p=P)
            NKT = S // P
            for i in range(4):
                nc.sync.dma_start(v_sb[:, bass.ts(i, NKT // 4), :],
                                  vv[:, bass.ts(i, NKT // 4), :])
            if after_loads is not None and b == 0 and h == 0:
                after_loads()
            for qt in range(S // QT):
                blocks = spec[qt]
                nb = len(blocks)
                assert nb % 2 == 0, "phase-2 pairing needs an even block count"
                psum_sum = psSum.tile([P, QT], F32, tag="sum")
                psum_ctx = psC.tile([P, QT], F32, tag="ctx")

                def mk_fini(ps_sum, ps_ctx, ctxT, h, qt):
                    def fini():
                        recip = pr.tile([P, QT], F32, tag="recip")
                        nc.vector.reciprocal(recip[:], ps_sum[:])
                        nc.vector.tensor_mul(
                            ctxT[:, h, bass.ds(qt * QT, QT)],
                            ps_ctx[:], recip[:])
                    return fini

                for pi in range(nb // 2):
                    (kt0, pat0), (kt1, pat1) = blocks[2 * pi], blocks[2 * pi + 1]
                    psum_s = psS.tile([P, 2, QT], F32, tag="s")
                    for i, kt in enumerate((kt0, kt1)):
                        nc.tensor.matmul(
                            psum_s[:, i, :], k_sb[:, bass.ts(kt, KT)],
                            q_sb[:, bass.ds(qt * QT, QT)],
                            start=True, stop=True)
                    for i, pat in enumerate((pat0, pat1)):
                        if pat is not None:
                            nc.vector.tensor_tensor(
                                psum_s[:, i, :], psum_s[:, i, :], mks[pat][:],
                                mybir.AluOpType.add)
                    e_sb = pe_.tile([P, 2, QT], BF, tag="e")
                    nc.scalar.activation(e_sb[:], psum_s[:], AF.Exp)
                    last = pi == nb // 2 - 1
                    pend.append((e_sb, v_sb, (kt0, kt1), psum_sum, psum_ctx,
                                 pi == 0, last,
                                 mk_fini(psum_sum, psum_ctx, ctxT, h, qt)
                                 if last else None))
                    if len(pend) > PIPE:
                        pop_one()
    while pend:
        pop_one()


def _phase3(nc, tc, pools, wo_r, ctx_tiles, ot):
    p3o, psO = pools
    OT = 512
    for b in range(B):
        ctxT = ctx_tiles[b]
        for oi in range(H // P):
            for qt in range(S // OT):
                psum_o = psO.tile([P, OT], F32, tag="o")
                for hs in range(NHL):
                    nc.tensor.matmul(
                        psum_o[:], wo_r[:, hs, bass.ts(oi, P)],
                        ctxT[:, hs, bass.ds(qt * OT, OT)],
                        start=(hs == 0), stop=(hs == NHL - 1))
                o_sb = p3o.tile([P, OT], DT, tag="o_sb")
                nc.scalar.activation(o_sb[:], psum_o[:], AF.Copy)
                nc.sync.dma_start(
                    ot[bass.ts(oi, P), bass.ds(b * S + qt * OT, OT)], o_sb[:])


def _build(specs, n_pat, reps=1, phases=(1, 2, 3)):
    nc = bacc.Bacc()

    xt = nc.declare_dram_parameter("xt", [H, BT], BF, isOutput=False)
    wqt = nc.declare_dram_parameter("wqt", [H, DL], BF, isOutput=False)
    wkt = nc.declare_dram_parameter("wkt", [H, DL], BF, isOutput=False)
    wvt = nc.declare_dram_parameter("wvt", [H, DL], BF, isOutput=False)
    wot = nc.declare_dram_parameter("wot", [DL, H], BF, isOutput=False)
    maskt = nc.declare_dram_parameter(
        "maskt", [max(n_pat, 1), P, QT], DT, isOutput=False)
    cost = nc.declare_dram_parameter("cost", [HD, S], BF, isOutput=False)
    sint = nc.declare_dram_parameter("sint", [HD, S], BF, isOutput=False)
    ot = nc.declare_dram_parameter("ot", [H, BT], DT, isOutput=True)

    xt3 = xt.rearrange("(ho p) t -> p ho t", p=P)
    wq3 = wqt.rearrange("(ho p) d -> p ho d", p=P)
    wk3 = wkt.rearrange("(ho p) d -> p ho d", p=P)
    wv3 = wvt.rearrange("(ho p) d -> p ho d", p=P)
    wo3 = wot.rearrange("(hs p) o -> p hs o", p=P)

    import contextlib

    with tile.TileContext(nc) as tc:
        with (
            tc.tile_pool(name="glob", bufs=1) as glob,
            tc.tile_pool(name="dram", bufs=1, space="DRAM") as dram,
        ):
            q_d = [dram.tile([DL, S], BF, tag=f"q_d{b}", name=f"q_d{b}")
                   for b in range(B)]
            k_d = [dram.tile([DL, S], BF, tag=f"k_d{b}", name=f"k_d{b}")
                   for b in range(B)]
            v_d = [dram.tile([S, DL], BF, tag=f"v_d{b}", name=f"v_d{b}")
                   for b in range(B)]

            ones_f = glob.tile([P, P], F32, tag="ones_f")
            nc.any.memset(ones_f[:], 1.0)
            ones_r = glob.tile([P, P], BF, tag="ones_r")
            nc.vector.tensor_copy(ones_r[:], ones_f[:])
            mks = []
            for i in range(n_pat):
                mk = glob.tile([P, QT], DT, tag=f"mk{i}", name=f"mk{i}")
                nc.sync.dma_start(mk[:], maskt[i])
                mks.append(mk)

            loop_cm = tc.For_i(0, reps, 1) if reps > 1 else contextlib.nullcontext()
            with loop_cm:
                if 1 in phases:
                    with (
                        tc.tile_pool(name="wpool", bufs=1) as wpool,
                        tc.tile_pool(name="xpool", bufs=2) as xpool,
                        tc.tile_pool(name="tabpool", bufs=1) as tabpool,
                        tc.tile_pool(name="rpool", bufs=3) as rpool,
                        tc.tile_pool(name="stpool", bufs=2) as stpool,
                        tc.tile_pool(name="vstpool", bufs=2) as vstpool,
                        tc.tile_pool(name="psA", bufs=3, space="PSUM") as psA,
                        tc.tile_pool(name="psV", bufs=4, space="PSUM") as psV,
                    ):
                        aps = (xt3, wq3, wk3, wv3, cost, sint)
                        _phase1(nc, tc,
                                (wpool, xpool, tabpool, rpool, stpool,
                                 vstpool, psA, psV),
                                aps, q_d, k_d, v_d)
                if 2 in phases:
                    with tc.tile_pool(name="ctxp", bufs=1) as ctxp:
                        ctx_tiles = []
                        with tc.tile_pool(name="p3w", bufs=1) as p3w:
                            # wo load issued after the first head's loads:
                            # overlaps phase 2 without delaying its start
                            wo_r = p3w.tile([P, NHL, H], BF, tag="wo_r")

                            def _load_wo():
                                nc.sync.dma_start(wo_r[:], wo3[:, :, :])
                            with (
                                tc.tile_pool(name="pk", bufs=3) as pk,
                                tc.tile_pool(name="pq", bufs=3) as pq,
                                tc.tile_pool(name="pv", bufs=3) as pv,
                                tc.tile_pool(name="pe", bufs=6) as pe_,
                                tc.tile_pool(name="pr", bufs=2) as pr,
                                tc.tile_pool(name="psS", bufs=4, space="PSUM") as psS,
                                tc.tile_pool(name="psSum", bufs=2, space="PSUM") as psSum,
                                tc.tile_pool(name="psC", bufs=2, space="PSUM") as psC,
                            ):
                                p2pools = (pk, pq, pv, pe_, pr, psS, psSum, psC)
                                for b in range(B):
                                    ctxT = ctxp.tile([P, NHL, S], BF,
                                                     tag=f"ctxT{b}",
                                                     name=f"ctxT{b}")
                                    ctx_tiles.append(ctxT)
                                _phase2(nc, tc, specs, p2pools, mks, ones_r,
                                        q_d, k_d, v_d, ctx_tiles,
                                        after_loads=_load_wo)
                            if 3 in phases:
                                with (
                                    tc.tile_pool(name="p3o", bufs=6) as p3o,
                                    tc.tile_pool(name="psO", bufs=6,
                                                 space="PSUM") as psO,
                                ):
                                    _phase3(nc, tc, (p3o, psO), wo_r,
                                            ctx_tiles, ot)
    nc.finalize()
    return nc


def _rope_tables():
    inv_freq = 1.0 / (10000.0 ** (np.arange(0, HD, 2, dtype=np.float32) / HD))
    t = np.arange(S, dtype=np.float32)
    freqs = np.einsum("i,j->ij", t, inv_freq)
    emb = np.concatenate([freqs, freqs], axis=-1)        # [S, HD]
    return np.cos(emb).astype(np.float32), np.sin(emb).astype(np.float32)


def _block_spec(mask, patterns, pat_idx):
    """mask: [S, S] additive (q, k). Returns per-qt list of (kt, pat|None).
    Partially-masked blocks get an index into the shared `patterns` list
    (each a [KT, QT] f32 array in [k, q] layout)."""
    spec = []
    for qt in range(S // QT):
        row = []
        sub_q = mask[qt * QT:(qt + 1) * QT]
        for kt in range(S // KT):
            blk = sub_q[:, kt * KT:(kt + 1) * KT]
            if np.all(blk <= -1e8):
                continue                        # fully masked -> skip
            if np.any(blk != 0.0):
                t = np.ascontiguousarray(blk.T.astype(np.float32))
                key = t.tobytes()
                if key not in pat_idx:
                    pat_idx[key] = len(patterns)
                    patterns.append(t)
                row.append((kt, pat_idx[key]))
            else:
                row.append((kt, None))
        assert row, "a query tile with all keys masked is not supported"
        spec.append(row)
    return spec


def prepare(hidden_states, attention_mask, Wq, Wk, Wv, Wo):
    """Host-side marshaling -> (in_maps, specs, n_pat)."""
    import ml_dtypes

    bf16 = ml_dtypes.bfloat16
    hidden_states = np.asarray(hidden_states, dtype=np.float32)
    attention_mask = np.asarray(attention_mask, dtype=np.float32)
    Wq = np.asarray(Wq, dtype=np.float32)
    Wk = np.asarray(Wk, dtype=np.float32)
    Wv = np.asarray(Wv, dtype=np.float32)
    Wo = np.asarray(Wo, dtype=np.float32)

    xbf = np.ascontiguousarray(
        hidden_states.reshape(BT, H).T).astype(bf16)            # [H, BT]
    scale = 1.0 / np.sqrt(np.float32(HD))
    wqT = np.ascontiguousarray(Wq.T * scale).astype(bf16)       # [H, H]
    wkT = np.ascontiguousarray(Wk.T).astype(bf16)
    wvT = np.ascontiguousarray(Wv.T).astype(bf16)
    woT = np.ascontiguousarray(Wo.T).astype(bf16)               # [H(in'), H(out)]

    masks = attention_mask[:, 0]                                # [B, S, S]
    same = bool(np.array_equal(masks[0], masks[1])) if B == 2 else True
    n_mb = 1 if same else B
    patterns, pat_idx = [], {}
    specs = [_block_spec(masks[i], patterns, pat_idx) for i in range(n_mb)]
    n_pat = len(patterns)
    if n_pat:
        maskt = np.ascontiguousarray(np.stack(patterns))        # [n_pat, P, QT]
    else:
        maskt = np.zeros((1, P, QT), dtype=np.float32)

    cos, sin = _rope_tables()
    cost = np.ascontiguousarray(cos.T).astype(bf16)             # [HD, S]
    sint = np.ascontiguousarray(sin.T).astype(bf16)

    in_maps = []
    for g in range(NC):
        dsl = slice(g * DL, (g + 1) * DL)
        in_maps.append({
            "xt": xbf,
            "wqt": np.ascontiguousarray(wqT[:, dsl]),
            "wkt": np.ascontiguousarray(wkT[:, dsl]),
            "wvt": np.ascontiguousarray(wvT[:, dsl]),
            "wot": np.ascontiguousarray(woT[dsl, :]),
            "maskt": maskt,
            "cost": cost, "sint": sint,
        })
    return in_maps, specs, n_pat


_CACHE = {}


def kernel(hidden_states, attention_mask, Wq, Wk, Wv, Wo):
    from concourse.bass_utils import run_bass_kernel_spmd

    in_maps, specs, n_pat = prepare(
        hidden_states, attention_mask, Wq, Wk, Wv, Wo)

    key = tuple(tuple(tuple(map(tuple, s)) for s in sp) for sp in specs)
    if key not in _CACHE:
        _CACHE[key] = _build(specs, n_pat)
    nc = _CACHE[key]

    try:
        res = run_bass_kernel_spmd(nc, in_maps, list(range(NC)), trace=False)
    except Exception:
        # one retry: a wedged NeuronCore usually recovers on re-dispatch
        import time as _time
        _time.sleep(5)
        res = run_bass_kernel_spmd(nc, in_maps, list(range(NC)), trace=False)
    acc = np.zeros((H, BT), dtype=np.float32)
    for g in range(NC):
        acc += res.results[g]["ot"]
    return np.ascontiguousarray(acc.T).reshape(B, S, H)


# revision 4
# speedup vs baseline: 1.2502x; 1.2227x over previous
"""LlamaAttention (B=2, S=2048, H=4096, NH=32) on 8 Trainium2 NeuronCores.

Sharding: tensor-parallel over heads (4 heads / core). Column-parallel
Wq/Wk/Wv, row-parallel Wo; the Wo partial sums are reduced on the host.

v2 design (vs baseline): weights are pre-cast to bf16 on the HOST and kept
RESIDENT in SBUF for all of phase 1 (no per-slice reloads); x is pre-cast
to bf16 on the host (halves streaming); the causal mask is applied from a
couple of small resident patterns instead of 32 MB of mask DMA; the score
scale 1/sqrt(HD) is folded into Wq on the host; phase-2 exp is
software-pipelined one block ahead so the PE never waits on ACT.

Per-core dataflow (all matmuls bf16 x bf16 -> f32 PSUM; exp output is
bf16 so numerator and denominator share the same rounding):
  phase 1 (8 chunks of 512 tokens):
      Q^T,K^T = RoPE(W^T @ X^T-chunk) -> DRAM [d, t] bf16
      V       = X^T-chunk^T @ WvT     -> DRAM [t, d] bf16
  phase 2 per (batch, head): S^T[k,q] = K-tile^T @ Q (contraction d=128),
      exp on ACT (block-pipelined); denominators via ones-matmul
      (partition-broadcast column sums); ctx^T[d,q] = V-tile^T @ expS^T.
      Only not-fully-masked 128x256 blocks are computed; partially masked
      blocks add a resident mask pattern.
  phase 3: O^T partial = WoT^T @ ctx^T -> DRAM [o, t] f32.

Host side: bf16 pre-casts, transposes, mask pattern extraction, sums the
8 partial O^T outputs and transposes back.
"""
import sys

sys.path.insert(0, "/opt/trn_rl_repo")

import numpy as np

import concourse.bass as bass
import concourse.bacc as bacc
import concourse.tile as tile
import concourse.mybir as mybir

B, S, H, NH = 2, 2048, 4096, 32
HD = H // NH          # 128
NC = 8                # cores
DL = H // NC          # 512 local dims (4 heads / core)
NHL = NH // NC        # 4 local heads
BT = B * S            # 4096 tokens
P = 128
CH = 512              # phase-1 x chunk (tokens)
QT = 256              # phase-2 query tile (free dim)
KT = 128              # phase-2 key tile (partition dim)
NKO = H // P          # 32 contraction subtiles
NW = 8                # weight load pieces (hs-sliced)

DT = mybir.dt.float32
DTR = mybir.dt.float32r
BF = mybir.dt.bfloat16
F32 = mybir.dt.float32
AF = mybir.ActivationFunctionType


def _phase1(nc, tc, pools, aps, q_d, k_d, v_d):
    (wpool, xpool, tabpool, rpool, stpool, vstpool, psA, psV) = pools
    xt3, wq3, wk3, wv3, cost, sint = aps

    HS = NKO // NW
    # startup order: interleave x-chunk-0 pieces with wv pieces so the PE
    # can start on chunk 0's V matmuls (which need no tables) within a few
    # microseconds; tables and wq/wk stream in under chunk 0's V compute.
    xc0 = xpool.tile([P, NKO, CH], BF, tag="xt", name="xc0")
    wps = {"wq": [], "wk": [], "wv": []}
    for i in range(NW):
        nc.sync.dma_start(xc0[:, bass.ts(i, HS), :],
                          xt3[:, bass.ts(i, HS), bass.ds(0, CH)])
        t = wpool.tile([P, HS, DL], BF, tag=f"wv{i}", name=f"wv{i}")
        nc.sync.dma_start(t[:], wv3[:, bass.ts(i, HS), :])
        wps["wv"].append(t)
    cosT = tabpool.tile([P, S], BF, tag="cosT")
    nc.sync.dma_start(cosT[:], cost[:, :])
    sinT = tabpool.tile([P, S], BF, tag="sinT")
    nc.sync.dma_start(sinT[:], sint[:, :])
    for nm, w3 in (("wq", wq3), ("wk", wk3)):
        for i in range(NW):
            t = wpool.tile([P, HS, DL], BF, tag=f"{nm}{i}", name=f"{nm}{i}")
            nc.sync.dma_start(t[:], w3[:, bass.ts(i, HS), :])
            wps[nm].append(t)

    xch = xc0
    for c in range(BT // CH):
        cb = (c * CH) // S                  # batch this chunk belongs to
        co = (c * CH) % S                   # column offset within batch
        tsl = bass.ds(co, CH)
        # --- V in [t, d] layout ---
        w_p = wps["wv"]
        vst = vstpool.tile([P, CH // P, DL], BF, tag="vst")
        psums = [psV.tile([P, DL], F32, tag="v", name=f"vps{c}_{j}")
                 for j in range(CH // P)]
        for hs in range(NKO):
            for j in range(CH // P):
                nc.tensor.matmul(
                    psums[j][:], xch[:, hs, bass.ts(j, P)],
                    w_p[hs // HS][:, hs % HS, :],
                    start=(hs == 0), stop=(hs == NKO - 1))
        for j in range(CH // P):
            nc.scalar.activation(vst[:, j, :], psums[j][:], AF.Copy)
        nc.sync.dma_start(
            v_d[cb][bass.ds(co, CH), :].rearrange("(j p) d -> p j d", p=P),
            vst[:])
        # next chunk prefetch, after the V loads/stores of this chunk
        if c + 1 < BT // CH:
            xn = xpool.tile([P, NKO, CH], BF, tag="xt", name=f"xc{c+1}")
            nc.sync.dma_start(xn[:], xt3[:, :, bass.ds((c + 1) * CH, CH)])
        else:
            xn = None
        # --- Q^T and K^T with RoPE ---
        for (nm, outd) in (("wq", q_d[cb]), ("wk", k_d[cb])):
            w_p = wps[nm]
            qst = stpool.tile([P, DL // P, CH], BF, tag="qst")
            for dsub in range(DL // P):
                psum = psA.tile([P, CH], F32, tag="qk")
                for hs in range(NKO):
                    nc.tensor.matmul(
                        psum[:], w_p[hs // HS][:, hs % HS, bass.ts(dsub, P)],
                        xch[:, hs, :], start=(hs == 0), stop=(hs == NKO - 1))
                rc = rpool.tile([P, CH], F32, tag="rc")
                rs = rpool.tile([P, CH], F32, tag="rs")
                nc.vector.tensor_mul(rc[:], psum[:], cosT[:, tsl])
                nc.vector.tensor_mul(rs[0:64, :], psum[64:128, :], sinT[0:64, tsl])
                nc.vector.tensor_mul(rs[64:128, :], psum[0:64, :], sinT[64:128, tsl])
                nc.vector.tensor_tensor(
                    qst[0:64, dsub, :], rc[0:64, :], rs[0:64, :],
                    mybir.AluOpType.subtract)
                nc.vector.tensor_tensor(
                    qst[64:128, dsub, :], rc[64:128, :], rs[64:128, :],
                    mybir.AluOpType.add)
            nc.sync.dma_start(
                outd[:, tsl].rearrange("(ds p) t -> p ds t", p=P), qst[:])
        xch = xn


PIPE = 2  # exp pipeline depth, in block PAIRS (ACT latency ~0.6us ~= 3 scores)


def _phase2(nc, tc, specs, pools, mks, ones_r, q_d, k_d, v_d, ctx_tiles,
            after_loads=None):
    """All (batch, head) attention with one exp pipeline flattened across
    qt/head/batch boundaries, so the PE never drains waiting on ACT.
    Adjacent kt blocks are PAIRED into one [P, 2, QT] PSUM bank so a single
    exp covers both (the ACT fixed access cost ~185ns is per instruction)."""
    (pk, pq, pv, pe_, pr, psS, psSum, psC) = pools

    pend = []  # (e_sb, v_sb, (kt0, kt1), psum_sum, psum_ctx, start, stop, fini)

    def pop_one():
        e_sb, v_sb, kts, ps_sum, ps_ctx, st, sp, fini = pend.pop(0)
        last = len(kts) - 1
        for i, kt in enumerate(kts):
            nc.tensor.matmul(ps_sum[:], ones_r[:], e_sb[:, i, :],
                             start=(st and i == 0), stop=(sp and i == last))
            nc.tensor.matmul(ps_ctx[:], v_sb[:, kt, :], e_sb[:, i, :],
                             start=(st and i == 0), stop=(sp and i == last))
        if fini is not None:
            fini()

    for b in range(B):
        spec = specs[b % len(specs)]
        ctxT = ctx_tiles[b]
        for h in range(NHL):
            k_sb = pk.tile([P, S], BF, tag="k_sb")
            for i in range(2):
                nc.sync.dma_start(k_sb[:, bass.ts(i, S // 2)],
                                  k_d[b][bass.ts(h, P), bass.ts(i, S // 2)])
            q_sb = pq.tile([P, S], BF, tag="q_sb")
            for i in range(2):
                nc.sync.dma_start(q_sb[:, bass.ts(i, S // 2)],
                                  q_d[b][bass.ts(h, P), bass.ts(i, S // 2)])
            v_sb = pv.tile([P, S // P, P], BF, tag="v_sb")
            vv = v_d[b][:, bass.ts(h, P)].rearrange("(kt p) d -> p kt d", p=P)
            NKT = S // P
            for i in range(4):
                nc.sync.dma_start(v_sb[:, bass.ts(i, NKT // 4), :],
                                  vv[:, bass.ts(i, NKT // 4), :])
            if after_loads is not None and b == 0 and h == 0:
                after_loads()
            for qt in range(S // QT):
                blocks = spec[qt]
                nb = len(blocks)
                psum_sum = psSum.tile([P, QT], F32, tag="sum")
                psum_ctx = psC.tile([P, QT], F32, tag="ctx")

                def mk_fini(ps_sum, ps_ctx, ctxT, h, qt):
                    def fini():
                        recip = pr.tile([P, QT], F32, tag="recip")
                        nc.vector.reciprocal(recip[:], ps_sum[:])
                        nc.vector.tensor_mul(
                            ctxT[:, h, bass.ds(qt * QT, QT)],
                            ps_ctx[:], recip[:])
                    return fini

                groups = [blocks[i:i + 2] for i in range(0, nb, 2)]
                for pi, grp in enumerate(groups):
                    ng = len(grp)
                    psum_s = psS.tile([P, 2, QT], F32, tag="s")
                    for i, (kt, pat) in enumerate(grp):
                        nc.tensor.matmul(
                            psum_s[:, i, :], k_sb[:, bass.ts(kt, KT)],
                            q_sb[:, bass.ds(qt * QT, QT)],
                            start=True, stop=True)
                        if pat is not None:
                            nc.vector.tensor_tensor(
                                psum_s[:, i, :], psum_s[:, i, :], mks[pat][:],
                                mybir.AluOpType.add)
                    e_sb = pe_.tile([P, 2, QT], BF, tag="e")
                    nc.scalar.activation(e_sb[:, 0:ng, :], psum_s[:, 0:ng, :],
                                         AF.Exp)
                    last = pi == len(groups) - 1
                    pend.append((e_sb, v_sb, tuple(kt for kt, _ in grp),
                                 psum_sum, psum_ctx, pi == 0, last,
                                 mk_fini(psum_sum, psum_ctx, ctxT, h, qt)
                                 if last else None))
                    if len(pend) > PIPE:
                        pop_one()
    while pend:
        pop_one()


def _phase3(nc, tc, pools, wo_r, ctx_tiles, ot):
    p3o, psO = pools
    OT = 512
    for b in range(B):
        ctxT = ctx_tiles[b]
        for oi in range(H // P):
            for qt in range(S // OT):
                psum_o = psO.tile([P, OT], F32, tag="o")
                for hs in range(NHL):
                    nc.tensor.matmul(
                        psum_o[:], wo_r[:, hs, bass.ts(oi, P)],
                        ctxT[:, hs, bass.ds(qt * OT, OT)],
                        start=(hs == 0), stop=(hs == NHL - 1))
                o_sb = p3o.tile([P, OT], DT, tag="o_sb")
                nc.scalar.activation(o_sb[:], psum_o[:], AF.Copy)
                nc.sync.dma_start(
                    ot[bass.ts(oi, P), bass.ds(b * S + qt * OT, OT)], o_sb[:])


def _build(specs, n_pat, reps=1, phases=(1, 2, 3)):
    nc = bacc.Bacc()

    xt = nc.declare_dram_parameter("xt", [H, BT], BF, isOutput=False)
    wqt = nc.declare_dram_parameter("wqt", [H, DL], BF, isOutput=False)
    wkt = nc.declare_dram_parameter("wkt", [H, DL], BF, isOutput=False)
    wvt = nc.declare_dram_parameter("wvt", [H, DL], BF, isOutput=False)
    wot = nc.declare_dram_parameter("wot", [DL, H], BF, isOutput=False)
    maskt = nc.declare_dram_parameter(
        "maskt", [max(n_pat, 1), P, QT], DT, isOutput=False)
    cost = nc.declare_dram_parameter("cost", [HD, S], BF, isOutput=False)
    sint = nc.declare_dram_parameter("sint", [HD, S], BF, isOutput=False)
    ot = nc.declare_dram_parameter("ot", [H, BT], DT, isOutput=True)

    xt3 = xt.rearrange("(ho p) t -> p ho t", p=P)
    wq3 = wqt.rearrange("(ho p) d -> p ho d", p=P)
    wk3 = wkt.rearrange("(ho p) d -> p ho d", p=P)
    wv3 = wvt.rearrange("(ho p) d -> p ho d", p=P)
    wo3 = wot.rearrange("(hs p) o -> p hs o", p=P)

    import contextlib

    with tile.TileContext(nc) as tc:
        with (
            tc.tile_pool(name="glob", bufs=1) as glob,
            tc.tile_pool(name="dram", bufs=1, space="DRAM") as dram,
        ):
            q_d = [dram.tile([DL, S], BF, tag=f"q_d{b}", name=f"q_d{b}")
                   for b in range(B)]
            k_d = [dram.tile([DL, S], BF, tag=f"k_d{b}", name=f"k_d{b}")
                   for b in range(B)]
            v_d = [dram.tile([S, DL], BF, tag=f"v_d{b}", name=f"v_d{b}")
                   for b in range(B)]

            ones_f = glob.tile([P, P], F32, tag="ones_f")
            nc.any.memset(ones_f[:], 1.0)
            ones_r = glob.tile([P, P], BF, tag="ones_r")
            nc.vector.tensor_copy(ones_r[:], ones_f[:])
            mks = []
            for i in range(n_pat):
                mk = glob.tile([P, QT], DT, tag=f"mk{i}", name=f"mk{i}")
                nc.sync.dma_start(mk[:], maskt[i])
                mks.append(mk)

            loop_cm = tc.For_i(0, reps, 1) if reps > 1 else contextlib.nullcontext()
            with loop_cm:
                if 1 in phases:
                    with (
                        tc.tile_pool(name="wpool", bufs=1) as wpool,
                        tc.tile_pool(name="xpool", bufs=2) as xpool,
                        tc.tile_pool(name="tabpool", bufs=1) as tabpool,
                        tc.tile_pool(name="rpool", bufs=3) as rpool,
                        tc.tile_pool(name="stpool", bufs=2) as stpool,
                        tc.tile_pool(name="vstpool", bufs=2) as vstpool,
                        tc.tile_pool(name="psA", bufs=3, space="PSUM") as psA,
                        tc.tile_pool(name="psV", bufs=4, space="PSUM") as psV,
                    ):
                        aps = (xt3, wq3, wk3, wv3, cost, sint)
                        _phase1(nc, tc,
                                (wpool, xpool, tabpool, rpool, stpool,
                                 vstpool, psA, psV),
                                aps, q_d, k_d, v_d)
                if 2 in phases:
                    with tc.tile_pool(name="ctxp", bufs=1) as ctxp:
                        ctx_tiles = []
                        with tc.tile_pool(name="p3w", bufs=1) as p3w:
                            # wo load issued after the first head's loads:
                            # overlaps phase 2 without delaying its start
                            wo_r = p3w.tile([P, NHL, H], BF, tag="wo_r")

                            def _load_wo():
                                nc.sync.dma_start(wo_r[:], wo3[:, :, :])
                            with (
                                tc.tile_pool(name="pk", bufs=3) as pk,
                                tc.tile_pool(name="pq", bufs=3) as pq,
                                tc.tile_pool(name="pv", bufs=3) as pv,
                                tc.tile_pool(name="pe", bufs=6) as pe_,
                                tc.tile_pool(name="pr", bufs=2) as pr,
                                tc.tile_pool(name="psS", bufs=4, space="PSUM") as psS,
                                tc.tile_pool(name="psSum", bufs=2, space="PSUM") as psSum,
                                tc.tile_pool(name="psC", bufs=2, space="PSUM") as psC,
                            ):
                                p2pools = (pk, pq, pv, pe_, pr, psS, psSum, psC)
                                for b in range(B):
                                    ctxT = ctxp.tile([P, NHL, S], BF,
                                                     tag=f"ctxT{b}",
                                                     name=f"ctxT{b}")
                                    ctx_tiles.append(ctxT)
                                _phase2(nc, tc, specs, p2pools, mks, ones_r,
                                        q_d, k_d, v_d, ctx_tiles,
                                        after_loads=_load_wo)
                            if 3 in phases:
                                with (
                                    tc.tile_pool(name="p3o", bufs=6) as p3o,
                                    tc.tile_pool(name="psO", bufs=6,
                                                 space="PSUM") as psO,
                                ):
                                    _phase3(nc, tc, (p3o, psO), wo_r,
                                            ctx_tiles, ot)
    nc.finalize()
    return nc


def _rope_tables():
    inv_freq = 1.0 / (10000.0 ** (np.arange(0, HD, 2, dtype=np.float32) / HD))
    t = np.arange(S, dtype=np.float32)
    freqs = np.einsum("i,j->ij", t, inv_freq)
    emb = np.concatenate([freqs, freqs], axis=-1)        # [S, HD]
    return np.cos(emb).astype(np.float32), np.sin(emb).astype(np.float32)


def _block_spec(mask, patterns, pat_idx):
    """mask: [S, S] additive (q, k). Returns per-qt list of (kt, pat|None).
    Partially-masked blocks get an index into the shared `patterns` list
    (each a [KT, QT] f32 array in [k, q] layout)."""
    spec = []
    for qt in range(S // QT):
        row = []
        sub_q = mask[qt * QT:(qt + 1) * QT]
        for kt in range(S // KT):
            blk = sub_q[:, kt * KT:(kt + 1) * KT]
            if np.all(blk <= -1e8):
                continue                        # fully masked -> skip
            if np.any(blk != 0.0):
                t = np.ascontiguousarray(blk.T.astype(np.float32))
                key = t.tobytes()
                if key not in pat_idx:
                    pat_idx[key] = len(patterns)
                    patterns.append(t)
                row.append((kt, pat_idx[key]))
            else:
                row.append((kt, None))
        assert row, "a query tile with all keys masked is not supported"
        spec.append(row)
    return spec


def prepare(hidden_states, attention_mask, Wq, Wk, Wv, Wo):
    """Host-side marshaling -> (in_maps, specs, n_pat)."""
    import ml_dtypes

    bf16 = ml_dtypes.bfloat16
    hidden_states = np.asarray(hidden_states, dtype=np.float32)
    attention_mask = np.asarray(attention_mask, dtype=np.float32)
    Wq = np.asarray(Wq, dtype=np.float32)
    Wk = np.asarray(Wk, dtype=np.float32)
    Wv = np.asarray(Wv, dtype=np.float32)
    Wo = np.asarray(Wo, dtype=np.float32)

    xbf = np.ascontiguousarray(
        hidden_states.reshape(BT, H).T).astype(bf16)            # [H, BT]
    scale = 1.0 / np.sqrt(np.float32(HD))
    wqT = np.ascontiguousarray(Wq.T * scale).astype(bf16)       # [H, H]
    wkT = np.ascontiguousarray(Wk.T).astype(bf16)
    wvT = np.ascontiguousarray(Wv.T).astype(bf16)
    woT = np.ascontiguousarray(Wo.T).astype(bf16)               # [H(in'), H(out)]

    masks = attention_mask[:, 0]                                # [B, S, S]
    same = bool(np.array_equal(masks[0], masks[1])) if B == 2 else True
    n_mb = 1 if same else B
    patterns, pat_idx = [], {}
    specs = [_block_spec(masks[i], patterns, pat_idx) for i in range(n_mb)]
    n_pat = len(patterns)
    if n_pat:
        maskt = np.ascontiguousarray(np.stack(patterns))        # [n_pat, P, QT]
    else:
        maskt = np.zeros((1, P, QT), dtype=np.float32)

    cos, sin = _rope_tables()
    cost = np.ascontiguousarray(cos.T).astype(bf16)             # [HD, S]
    sint = np.ascontiguousarray(sin.T).astype(bf16)

    in_maps = []
    for g in range(NC):
        dsl = slice(g * DL, (g + 1) * DL)
        in_maps.append({
            "xt": xbf,
            "wqt": np.ascontiguousarray(wqT[:, dsl]),
            "wkt": np.ascontiguousarray(wkT[:, dsl]),
            "wvt": np.ascontiguousarray(wvT[:, dsl]),
            "wot": np.ascontiguousarray(woT[dsl, :]),
            "maskt": maskt,
            "cost": cost, "sint": sint,
        })
    return in_maps, specs, n_pat


_CACHE = {}


def kernel(hidden_states, attention_mask, Wq, Wk, Wv, Wo):
    from concourse.bass_utils import run_bass_kernel_spmd

    in_maps, specs, n_pat = prepare(
        hidden_states, attention_mask, Wq, Wk, Wv, Wo)

    key = tuple(tuple(tuple(map(tuple, s)) for s in sp) for sp in specs)
    if key not in _CACHE:
        _CACHE[key] = _build(specs, n_pat)
    nc = _CACHE[key]

    try:
        res = run_bass_kernel_spmd(nc, in_maps, list(range(NC)), trace=False)
    except Exception:
        # one retry: a wedged NeuronCore usually recovers on re-dispatch
        import time as _time
        _time.sleep(5)
        res = run_bass_kernel_spmd(nc, in_maps, list(range(NC)), trace=False)
    acc = np.zeros((H, BT), dtype=np.float32)
    for g in range(NC):
        acc += res.results[g]["ot"]
    return np.ascontiguousarray(acc.T).reshape(B, S, H)


# revision 7
# speedup vs baseline: 1.2962x; 1.0368x over previous
"""LlamaAttention (B=2, S=2048, H=4096, NH=32) on 8 Trainium2 NeuronCores.

Sharding: tensor-parallel over heads (4 heads / core). Column-parallel
Wq/Wk/Wv, row-parallel Wo; the Wo partial sums are reduced on the host.

v2 design (vs baseline): weights are pre-cast to bf16 on the HOST and kept
RESIDENT in SBUF for all of phase 1 (no per-slice reloads); x is pre-cast
to bf16 on the host (halves streaming); the causal mask is applied from a
couple of small resident patterns instead of 32 MB of mask DMA; the score
scale 1/sqrt(HD) is folded into Wq on the host; phase-2 exp is
software-pipelined one block ahead so the PE never waits on ACT.

Per-core dataflow (all matmuls bf16 x bf16 -> f32 PSUM; exp output is
bf16 so numerator and denominator share the same rounding):
  phase 1 (8 chunks of 512 tokens):
      Q^T,K^T = RoPE(W^T @ X^T-chunk) -> DRAM [d, t] bf16
      V       = X^T-chunk^T @ WvT     -> DRAM [t, d] bf16
  phase 2 per (batch, head): S^T[k,q] = K-tile^T @ Q (contraction d=128),
      exp on ACT (block-pipelined); denominators via ones-matmul
      (partition-broadcast column sums); ctx^T[d,q] = V-tile^T @ expS^T.
      Only not-fully-masked 128x256 blocks are computed; partially masked
      blocks add a resident mask pattern.
  phase 3: O^T partial = WoT^T @ ctx^T -> DRAM [o, t] f32.

Host side: bf16 pre-casts, transposes, mask pattern extraction, sums the
8 partial O^T outputs and transposes back.
"""
import sys

sys.path.insert(0, "/opt/trn_rl_repo")

import numpy as np

import concourse.bass as bass
import concourse.bacc as bacc
import concourse.tile as tile
import concourse.mybir as mybir

B, S, H, NH = 2, 2048, 4096, 32
HD = H // NH          # 128
NC = 8                # cores
DL = H // NC          # 512 local dims (4 heads / core)
NHL = NH // NC        # 4 local heads
BT = B * S            # 4096 tokens
P = 128
CH = 512              # phase-1 x chunk (tokens)
QT = 256              # phase-2 query tile (free dim)
KT = 128              # phase-2 key tile (partition dim)
NKO = H // P          # 32 contraction subtiles
NW = 8                # weight load pieces (hs-sliced)

DT = mybir.dt.float32
DTR = mybir.dt.float32r
BF = mybir.dt.bfloat16
F32 = mybir.dt.float32
AF = mybir.ActivationFunctionType


def _phase1(nc, tc, pools, aps, q_d, k_d, v_d):
    (wpool, xpool, tabpool, rpool, stpool, vstpool, psA, psV) = pools
    xt3, wq3, wk3, wv3, cost, sint = aps

    HS = NKO // NW
    # startup order: interleave x-chunk-0 pieces with wv pieces so the PE
    # can start on chunk 0's V matmuls (which need no tables) within a few
    # microseconds; tables and wq/wk stream in under chunk 0's V compute.
    xc0 = xpool.tile([P, NKO, CH], BF, tag="xt", name="xc0")
    wps = {"wq": [], "wk": [], "wv": []}
    for i in range(NW):
        nc.sync.dma_start(xc0[:, bass.ts(i, HS), :],
                          xt3[:, bass.ts(i, HS), bass.ds(0, CH)])
        t = wpool.tile([P, HS, DL], BF, tag=f"wv{i}", name=f"wv{i}")
        nc.sync.dma_start(t[:], wv3[:, bass.ts(i, HS), :])
        wps["wv"].append(t)
    cosT = tabpool.tile([P, S], BF, tag="cosT")
    nc.sync.dma_start(cosT[:], cost[:, :])
    sinT = tabpool.tile([P, S], BF, tag="sinT")
    nc.sync.dma_start(sinT[:], sint[:, :])
    for nm, w3 in (("wq", wq3), ("wk", wk3)):
        for i in range(NW):
            t = wpool.tile([P, HS, DL], BF, tag=f"{nm}{i}", name=f"{nm}{i}")
            nc.sync.dma_start(t[:], w3[:, bass.ts(i, HS), :])
            wps[nm].append(t)

    xch = xc0
    for c in range(BT // CH):
        cb = (c * CH) // S                  # batch this chunk belongs to
        co = (c * CH) % S                   # column offset within batch
        tsl = bass.ds(co, CH)
        # --- V in [t, d] layout ---
        w_p = wps["wv"]
        vst = vstpool.tile([P, CH // P, DL], BF, tag="vst")
        psums = [psV.tile([P, DL], F32, tag="v", name=f"vps{c}_{j}")
                 for j in range(CH // P)]
        for hs in range(NKO):
            for j in range(CH // P):
                nc.tensor.matmul(
                    psums[j][:], xch[:, hs, bass.ts(j, P)],
                    w_p[hs // HS][:, hs % HS, :],
                    start=(hs == 0), stop=(hs == NKO - 1))
        for j in range(CH // P):
            nc.scalar.activation(vst[:, j, :], psums[j][:], AF.Copy)
        nc.sync.dma_start(
            v_d[cb][bass.ds(co, CH), :].rearrange("(j p) d -> p j d", p=P),
            vst[:])
        # next chunk prefetch, after the V loads/stores of this chunk
        if c + 1 < BT // CH:
            xn = xpool.tile([P, NKO, CH], BF, tag="xt", name=f"xc{c+1}")
            nc.sync.dma_start(xn[:], xt3[:, :, bass.ds((c + 1) * CH, CH)])
        else:
            xn = None
        # --- Q^T and K^T with RoPE ---
        for (nm, outd) in (("wq", q_d[cb]), ("wk", k_d[cb])):
            w_p = wps[nm]
            qst = stpool.tile([P, DL // P, CH], BF, tag="qst")
            for dsub in range(DL // P):
                psum = psA.tile([P, CH], F32, tag="qk")
                for hs in range(NKO):
                    nc.tensor.matmul(
                        psum[:], w_p[hs // HS][:, hs % HS, bass.ts(dsub, P)],
                        xch[:, hs, :], start=(hs == 0), stop=(hs == NKO - 1))
                rc = rpool.tile([P, CH], F32, tag="rc")
                rs = rpool.tile([P, CH], F32, tag="rs")
                nc.vector.tensor_mul(rc[:], psum[:], cosT[:, tsl])
                nc.vector.tensor_mul(rs[0:64, :], psum[64:128, :], sinT[0:64, tsl])
                nc.vector.tensor_mul(rs[64:128, :], psum[0:64, :], sinT[64:128, tsl])
                nc.vector.tensor_tensor(
                    qst[0:64, dsub, :], rc[0:64, :], rs[0:64, :],
                    mybir.AluOpType.subtract)
                nc.vector.tensor_tensor(
                    qst[64:128, dsub, :], rc[64:128, :], rs[64:128, :],
                    mybir.AluOpType.add)
            nc.sync.dma_start(
                outd[:, tsl].rearrange("(ds p) t -> p ds t", p=P), qst[:])
        xch = xn


PIPE = 2  # exp pipeline depth, in block PAIRS (ACT latency ~0.6us ~= 3 scores)


def _phase2(nc, tc, specs, pools, mks, ones_r, q_d, k_d, v_d, ctx_tiles,
            after_loads=None):
    """All (batch, head) attention with one exp pipeline flattened across
    qt/head/batch boundaries, so the PE never drains waiting on ACT.
    Adjacent kt blocks are PAIRED into one [P, 2, QT] PSUM bank so a single
    exp covers both (the ACT fixed access cost ~185ns is per instruction)."""
    (pk, pq, pv, pe_, pr, psS, psSum, psC) = pools

    pend = []  # (e_sb, v_sb, (kt0, kt1), psum_sum, psum_ctx, start, stop, fini)

    def pop_one():
        e_sb, v_sb, kts, ps_sum, ps_ctx, st, sp, fini = pend.pop(0)
        last = len(kts) - 1
        for i, kt in enumerate(kts):
            nc.tensor.matmul(ps_sum[:], ones_r[:], e_sb[:, i, :],
                             start=(st and i == 0), stop=(sp and i == last))
            nc.tensor.matmul(ps_ctx[:], v_sb[:, kt, :], e_sb[:, i, :],
                             start=(st and i == 0), stop=(sp and i == last))
        if fini is not None:
            fini()

    for b in range(B):
        spec = specs[b % len(specs)]
        ctxT = ctx_tiles[b]
        for h in range(NHL):
            k_sb = pk.tile([P, S], BF, tag="k_sb")
            for i in range(2):
                nc.sync.dma_start(k_sb[:, bass.ts(i, S // 2)],
                                  k_d[b][bass.ts(h, P), bass.ts(i, S // 2)])
            q_sb = pq.tile([P, S], BF, tag="q_sb")
            for i in range(2):
                nc.sync.dma_start(q_sb[:, bass.ts(i, S // 2)],
                                  q_d[b][bass.ts(h, P), bass.ts(i, S // 2)])
            v_sb = pv.tile([P, S // P, P], BF, tag="v_sb")
            vv = v_d[b][:, bass.ts(h, P)].rearrange("(kt p) d -> p kt d", p=P)
            NKT = S // P
            for i in range(4):
                nc.sync.dma_start(v_sb[:, bass.ts(i, NKT // 4), :],
                                  vv[:, bass.ts(i, NKT // 4), :])
            if after_loads is not None and b == 0 and h == 0:
                after_loads()
            for qt in range(S // QT):
                blocks = spec[qt]
                nb = len(blocks)
                psum_sum = psSum.tile([P, QT], F32, tag="sum")
                psum_ctx = psC.tile([P, QT], F32, tag="ctx")

                def mk_fini(ps_sum, ps_ctx, ctxT, h, qt):
                    def fini():
                        recip = pr.tile([P, QT], F32, tag="recip")
                        nc.vector.reciprocal(recip[:], ps_sum[:])
                        nc.vector.tensor_mul(
                            ctxT[:, h, bass.ds(qt * QT, QT)],
                            ps_ctx[:], recip[:])
                    return fini

                groups = [blocks[i:i + 2] for i in range(0, nb, 2)]
                for pi, grp in enumerate(groups):
                    ng = len(grp)
                    psum_s = psS.tile([P, 2, QT], F32, tag="s")
                    for i, (kt, pat) in enumerate(grp):
                        nc.tensor.matmul(
                            psum_s[:, i, :], k_sb[:, bass.ts(kt, KT)],
                            q_sb[:, bass.ds(qt * QT, QT)],
                            start=True, stop=True)
                        if pat is not None:
                            nc.vector.tensor_tensor(
                                psum_s[:, i, :], psum_s[:, i, :], mks[pat][:],
                                mybir.AluOpType.add)
                    e_sb = pe_.tile([P, 2, QT], BF, tag="e")
                    nc.scalar.activation(e_sb[:, 0:ng, :], psum_s[:, 0:ng, :],
                                         AF.Exp)
                    last = pi == len(groups) - 1
                    pend.append((e_sb, v_sb, tuple(kt for kt, _ in grp),
                                 psum_sum, psum_ctx, pi == 0, last,
                                 mk_fini(psum_sum, psum_ctx, ctxT, h, qt)
                                 if last else None))
                    if len(pend) > PIPE:
                        pop_one()
    while pend:
        pop_one()


def _phase3(nc, tc, pools, wo_r, ctx_tiles, ot):
    p3o, psO = pools
    OT = 512
    for b in range(B):
        ctxT = ctx_tiles[b]
        for oi in range(H // P):
            for qt in range(S // OT):
                psum_o = psO.tile([P, OT], F32, tag="o")
                for hs in range(NHL):
                    nc.tensor.matmul(
                        psum_o[:], wo_r[:, hs, bass.ts(oi, P)],
                        ctxT[:, hs, bass.ds(qt * OT, OT)],
                        start=(hs == 0), stop=(hs == NHL - 1))
                o_sb = p3o.tile([P, OT], DT, tag="o_sb")
                nc.scalar.activation(o_sb[:], psum_o[:], AF.Copy)
                nc.sync.dma_start(
                    ot[bass.ts(oi, P), bass.ds(b * S + qt * OT, OT)], o_sb[:])


def _build(specs, n_pat, reps=1, phases=(1, 2, 3)):
    nc = bacc.Bacc()

    xt = nc.declare_dram_parameter("xt", [H, BT], BF, isOutput=False)
    wqt = nc.declare_dram_parameter("wqt", [H, DL], BF, isOutput=False)
    wkt = nc.declare_dram_parameter("wkt", [H, DL], BF, isOutput=False)
    wvt = nc.declare_dram_parameter("wvt", [H, DL], BF, isOutput=False)
    wot = nc.declare_dram_parameter("wot", [DL, H], BF, isOutput=False)
    maskt = nc.declare_dram_parameter(
        "maskt", [max(n_pat, 1), P, QT], DT, isOutput=False)
    cost = nc.declare_dram_parameter("cost", [HD, S], BF, isOutput=False)
    sint = nc.declare_dram_parameter("sint", [HD, S], BF, isOutput=False)
    ot = nc.declare_dram_parameter("ot", [H, BT], DT, isOutput=True)

    xt3 = xt.rearrange("(ho p) t -> p ho t", p=P)
    wq3 = wqt.rearrange("(ho p) d -> p ho d", p=P)
    wk3 = wkt.rearrange("(ho p) d -> p ho d", p=P)
    wv3 = wvt.rearrange("(ho p) d -> p ho d", p=P)
    wo3 = wot.rearrange("(hs p) o -> p hs o", p=P)

    import contextlib

    with tile.TileContext(nc) as tc:
        with (
            tc.tile_pool(name="glob", bufs=1) as glob,
            tc.tile_pool(name="dram", bufs=1, space="DRAM") as dram,
        ):
            q_d = [dram.tile([DL, S], BF, tag=f"q_d{b}", name=f"q_d{b}")
                   for b in range(B)]
            k_d = [dram.tile([DL, S], BF, tag=f"k_d{b}", name=f"k_d{b}")
                   for b in range(B)]
            v_d = [dram.tile([S, DL], BF, tag=f"v_d{b}", name=f"v_d{b}")
                   for b in range(B)]

            ones_f = glob.tile([P, P], F32, tag="ones_f")
            nc.any.memset(ones_f[:], 1.0)
            ones_r = glob.tile([P, P], BF, tag="ones_r")
            nc.vector.tensor_copy(ones_r[:], ones_f[:])
            mks = []
            for i in range(n_pat):
                mk = glob.tile([P, QT], DT, tag=f"mk{i}", name=f"mk{i}")
                nc.sync.dma_start(mk[:], maskt[i])
                mks.append(mk)

            loop_cm = tc.For_i(0, reps, 1) if reps > 1 else contextlib.nullcontext()
            with loop_cm:
                if 1 in phases:
                    with (
                        tc.tile_pool(name="wpool", bufs=1) as wpool,
                        tc.tile_pool(name="xpool", bufs=2) as xpool,
                        tc.tile_pool(name="tabpool", bufs=1) as tabpool,
                        tc.tile_pool(name="rpool", bufs=3) as rpool,
                        tc.tile_pool(name="stpool", bufs=2) as stpool,
                        tc.tile_pool(name="vstpool", bufs=2) as vstpool,
                        tc.tile_pool(name="psA", bufs=3, space="PSUM") as psA,
                        tc.tile_pool(name="psV", bufs=4, space="PSUM") as psV,
                    ):
                        aps = (xt3, wq3, wk3, wv3, cost, sint)
                        _phase1(nc, tc,
                                (wpool, xpool, tabpool, rpool, stpool,
                                 vstpool, psA, psV),
                                aps, q_d, k_d, v_d)
                if 2 in phases:
                    with tc.tile_pool(name="ctxp", bufs=1) as ctxp:
                        ctx_tiles = []
                        with tc.tile_pool(name="p3w", bufs=1) as p3w:
                            # wo load issued after the first head's loads:
                            # overlaps phase 2 without delaying its start
                            wo_r = p3w.tile([P, NHL, H], BF, tag="wo_r")

                            def _load_wo():
                                nc.sync.dma_start(wo_r[:], wo3[:, :, :])
                            with (
                                tc.tile_pool(name="pk", bufs=3) as pk,
                                tc.tile_pool(name="pq", bufs=3) as pq,
                                tc.tile_pool(name="pv", bufs=3) as pv,
                                tc.tile_pool(name="pe", bufs=6) as pe_,
                                tc.tile_pool(name="pr", bufs=2) as pr,
                                tc.tile_pool(name="psS", bufs=4, space="PSUM") as psS,
                                tc.tile_pool(name="psSum", bufs=2, space="PSUM") as psSum,
                                tc.tile_pool(name="psC", bufs=2, space="PSUM") as psC,
                            ):
                                p2pools = (pk, pq, pv, pe_, pr, psS, psSum, psC)
                                for b in range(B):
                                    ctxT = ctxp.tile([P, NHL, S], BF,
                                                     tag=f"ctxT{b}",
                                                     name=f"ctxT{b}")
                                    ctx_tiles.append(ctxT)
                                _phase2(nc, tc, specs, p2pools, mks, ones_r,
                                        q_d, k_d, v_d, ctx_tiles,
                                        after_loads=_load_wo)
                            if 3 in phases:
                                with (
                                    tc.tile_pool(name="p3o", bufs=6) as p3o,
                                    tc.tile_pool(name="psO", bufs=6,
                                                 space="PSUM") as psO,
                                ):
                                    _phase3(nc, tc, (p3o, psO), wo_r,
                                            ctx_tiles, ot)
    nc.finalize()
    return nc


def _rope_tables():
    inv_freq = 1.0 / (10000.0 ** (np.arange(0, HD, 2, dtype=np.float32) / HD))
    t = np.arange(S, dtype=np.float32)
    freqs = np.einsum("i,j->ij", t, inv_freq)
    emb = np.concatenate([freqs, freqs], axis=-1)        # [S, HD]
    return np.cos(emb).astype(np.float32), np.sin(emb).astype(np.float32)


def _block_spec(mask, patterns, pat_idx):
    """mask: [S, S] additive (q, k). Returns per-qt list of (kt, pat|None).
    Partially-masked blocks get an index into the shared `patterns` list
    (each a [KT, QT] f32 array in [k, q] layout)."""
    spec = []
    for qt in range(S // QT):
        row = []
        sub_q = mask[qt * QT:(qt + 1) * QT]
        for kt in range(S // KT):
            blk = sub_q[:, kt * KT:(kt + 1) * KT]
            if np.all(blk <= -1e8):
                continue                        # fully masked -> skip
            if np.any(blk != 0.0):
                t = np.ascontiguousarray(blk.T.astype(np.float32))
                key = t.tobytes()
                if key not in pat_idx:
                    pat_idx[key] = len(patterns)
                    patterns.append(t)
                row.append((kt, pat_idx[key]))
            else:
                row.append((kt, None))
        assert row, "a query tile with all keys masked is not supported"
        spec.append(row)
    return spec


def prepare(hidden_states, attention_mask, Wq, Wk, Wv, Wo):
    """Host-side marshaling -> (in_maps, specs, n_pat)."""
    import ml_dtypes

    bf16 = ml_dtypes.bfloat16
    hidden_states = np.asarray(hidden_states, dtype=np.float32)
    attention_mask = np.asarray(attention_mask, dtype=np.float32)
    Wq = np.asarray(Wq, dtype=np.float32)
    Wk = np.asarray(Wk, dtype=np.float32)
    Wv = np.asarray(Wv, dtype=np.float32)
    Wo = np.asarray(Wo, dtype=np.float32)

    xbf = np.ascontiguousarray(
        hidden_states.reshape(BT, H).T).astype(bf16)            # [H, BT]
    scale = 1.0 / np.sqrt(np.float32(HD))
    wqT = np.ascontiguousarray(Wq.T * scale).astype(bf16)       # [H, H]
    wkT = np.ascontiguousarray(Wk.T).astype(bf16)
    wvT = np.ascontiguousarray(Wv.T).astype(bf16)
    woT = np.ascontiguousarray(Wo.T).astype(bf16)               # [H(in'), H(out)]

    masks = attention_mask[:, 0]                                # [B, S, S]
    same = bool(np.array_equal(masks[0], masks[1])) if B == 2 else True
    n_mb = 1 if same else B
    patterns, pat_idx = [], {}
    specs = [_block_spec(masks[i], patterns, pat_idx) for i in range(n_mb)]
    n_pat = len(patterns)
    if n_pat:
        maskt = np.ascontiguousarray(np.stack(patterns))        # [n_pat, P, QT]
    else:
        maskt = np.zeros((1, P, QT), dtype=np.float32)

    cos, sin = _rope_tables()
    cost = np.ascontiguousarray(cos.T).astype(bf16)             # [HD, S]
    sint = np.ascontiguousarray(sin.T).astype(bf16)

    in_maps = []
    for g in range(NC):
        dsl = slice(g * DL, (g + 1) * DL)
        in_maps.append({
            "xt": xbf,
            "wqt": np.ascontiguousarray(wqT[:, dsl]),
            "wkt": np.ascontiguousarray(wkT[:, dsl]),
            "wvt": np.ascontiguousarray(wvT[:, dsl]),
            "wot": np.ascontiguousarray(woT[dsl, :]),
            "maskt": maskt,
            "cost": cost, "sint": sint,
        })
    return in_maps, specs, n_pat


_CACHE = {}


def kernel(hidden_states, attention_mask, Wq, Wk, Wv, Wo):
    from concourse.bass_utils import run_bass_kernel_spmd

    in_maps, specs, n_pat = prepare(
        hidden_states, attention_mask, Wq, Wk, Wv, Wo)

    key = tuple(tuple(tuple(map(tuple, s)) for s in sp) for sp in specs)
    if key not in _CACHE:
        _CACHE[key] = _build(specs, n_pat)
    nc = _CACHE[key]

    try:
        res = run_bass_kernel_spmd(nc, in_maps, list(range(NC)), trace=False)
    except Exception:
        # one retry: a wedged NeuronCore usually recovers on re-dispatch
        import time as _time
        _time.sleep(5)
        res = run_bass_kernel_spmd(nc, in_maps, list(range(NC)), trace=False)
    acc = np.zeros((H, BT), dtype=np.float32)
    for g in range(NC):
        acc += res.results[g]["ot"]
    return np.ascontiguousarray(acc.T).reshape(B, S, H)
